# revision 6
# baseline (speedup 1.0000x reference)
"""Trainium2 Bass kernel for nn_KnowledgeAttention.

Math (per batch b):
    e = enc @ W1 + b1                    [T, U]
    k = know @ W2 + b2                   [K, U]
    score[t, k] = V . tanh(e[t] + k[k])  (+ bV, dropped: softmax-invariant)
    w = softmax_k(score)
    out[t] = sum_k w[t, k] * know[k]     [T, D]

Sharding: pure data-parallel over (B=4) x (T split in 2) = 8 shards, one per
NeuronCore.  Weights replicated.  No collectives.

Per-core design (U = 128 = partition dim for the big stage):
    e_sb [u, t], k_sb [u, k]  via PE matmuls on PE-transposed inputs.
    h [u, tblk*K] = k_sb broadcast-added with one e column per t
        (DVE tensor_scalar_add, per-partition scalar, 2x_2P fp32 mode).
    tanh: one big ACT op per block, bias = b1+b2 folded in free, out bf16.
    score: matmul with hb chunk as the STATIONARY operand and V as the
        1-column moving operand: out[:, col] = hb[:, c:c+128].T @ V -> 128
        partition-dense score values per matmul, all 512 columns accumulate
        into ONE PSUM bank laid out [k mod 128, (t, kh)].
    softmax over k without leaving that layout: exp (ACT, PSUM->SBUF,
        written as [p, (kh, t)]), sum over k via PE matmuls against a ones
        column, reciprocal on DVE, normalization fused into the context
        eviction (tensor_scalar_mul by 1/S per t-partition).
    context: PE matmul, lhsT = E chunk [k, t], rhs = know [k, d].
"""

import numpy as np

import concourse.tile as tile
from concourse import bacc, mybir
from concourse.bass_utils import run_bass_kernel_spmd
from concourse.masks import make_identity

B, T, K, D, U = 4, 512, 256, 256, 128
TL = T // 2  # timesteps per core
NCORES = 8
TBLK = 64  # timesteps per main-loop block
NBLK = TL // TBLK

F32 = mybir.dt.float32
BF16 = mybir.dt.bfloat16
AF = mybir.ActivationFunctionType


def _body(tc, enc, know, w1, b1, w2, b2, v, out, reps=1):
    nc = tc.nc
    with (
        tc.tile_pool(name="const", bufs=1) as cpool,
        tc.tile_pool(name="h", bufs=2) as hpool,
        tc.tile_pool(name="hb", bufs=2) as hbpool,
        tc.tile_pool(name="ps_tr", bufs=2, space="PSUM") as ps_tr,
        tc.tile_pool(name="ps_ek", bufs=2, space="PSUM") as ps_ek,
        tc.tile_pool(name="ps_sc", bufs=1, space="PSUM") as ps_sc,
        tc.tile_pool(name="ps_sum", bufs=2, space="PSUM") as ps_sum,
    ):
        # ---------------- persistent SBUF tiles ----------------
        enc_sb = cpool.tile([128, 2 * D], F32, tag="enc")  # [t-part, (tc, d)]
        know_sb = cpool.tile([128, 2 * D], F32, tag="know")  # [k-part, (kc, d)]
        w1_sb = cpool.tile([128, 2 * U], F32, tag="w1")  # [d-part, (dc, u)]
        w2_sb = cpool.tile([128, 2 * U], F32, tag="w2")
        encT = cpool.tile([128, 2 * TL], F32, tag="encT")  # [d-part, (dc, t)]
        knowT = cpool.tile([128, 2 * K], F32, tag="knowT")  # [d-part, (dc, k)]
        e_sb = cpool.tile([128, TL], F32, tag="e")  # [u, t]
        k_bf = cpool.tile([128, K], BF16, tag="kbf")  # [u, k]
        b1_sb = cpool.tile([128, 1], F32, tag="b1")
        b2_sb = cpool.tile([128, 1], F32, tag="b2")
        b12 = cpool.tile([128, 1], F32, tag="b12")
        v_sb = cpool.tile([128, 1], F32, tag="v")
        v_bf = cpool.tile([128, 1], BF16, tag="vbf")
        ones_sb = cpool.tile([128, 1], F32, tag="ones")
        ident = cpool.tile([128, 128], F32, tag="ident")
        e_exp = cpool.tile([128, 2 * K], F32, tag="eexp")  # [p, (kh, t)]

        # ---------------- loads ----------------
        for c in range(2):
            for q in range(4):
                nc.sync.dma_start(
                    know_sb[:, c * D + q * 64 : c * D + (q + 1) * 64],
                    know[c * 128 : (c + 1) * 128, q * 64 : (q + 1) * 64],
                )
            for q in range(4):
                nc.sync.dma_start(
                    enc_sb[:, c * D + q * 64 : c * D + (q + 1) * 64],
                    enc[c * 128 : (c + 1) * 128, q * 64 : (q + 1) * 64],
                )
            nc.sync.dma_start(w1_sb[:, c * U : (c + 1) * U], w1[c * 128 : (c + 1) * 128, :])
            nc.sync.dma_start(w2_sb[:, c * U : (c + 1) * U], w2[c * 128 : (c + 1) * 128, :])
        nc.sync.dma_start(b1_sb[:], b1[:, :])
        nc.sync.dma_start(b2_sb[:], b2[:, :])
        nc.sync.dma_start(v_sb[:], v[:, :])
        make_identity(nc, ident[:])
        nc.gpsimd.memset(ones_sb[:], 1.0)
        nc.vector.tensor_add(b12[:], b1_sb[:], b2_sb[:])
        nc.vector.tensor_copy(v_bf[:], v_sb[:])

        # ---------------- transposes: enc -> encT, know -> knowT ----------------
        for src_sb, dstT in ((enc_sb, encT), (know_sb, knowT)):
            for ci in range(2):  # source partition chunk (t or k)
                for dc in range(2):  # d chunk
                    pt = ps_tr.tile([128, 128], F32, tag="tr")
                    nc.tensor.transpose(
                        pt[:], src_sb[:, ci * D + dc * 128 : ci * D + dc * 128 + 128], ident[:]
                    )
                    nc.scalar.copy(dstT[:, dc * 256 + ci * 128 : dc * 256 + ci * 128 + 128], pt[:])

        # ---------------- e/k projection matmuls ----------------
        e_ps = ps_ek.tile([128, TL], F32, tag="ek")
        nc.tensor.matmul(e_ps[:], w1_sb[:, 0:U], encT[:, 0:TL], start=True, stop=False)
        nc.tensor.matmul(e_ps[:], w1_sb[:, U : 2 * U], encT[:, TL : 2 * TL], start=False, stop=True)
        nc.scalar.copy(e_sb[:], e_ps[:])
        k_ps = ps_ek.tile([128, K], F32, tag="ek")
        nc.tensor.matmul(k_ps[:], w2_sb[:, 0:U], knowT[:, 0:K], start=True, stop=False)
        nc.tensor.matmul(k_ps[:], w2_sb[:, U : 2 * U], knowT[:, K : 2 * K], start=False, stop=True)
        nc.scalar.copy(k_bf[:], k_ps[:])

        # ---------------- main loop: broadcast-add, tanh, score ----------------
        # score_ps[p, 2*t + kh] = score[t, kh*128 + p]; one bank holds all of it.
        score_ps = ps_sc.tile([128, 4 * TBLK * NBLK // 2], F32, tag="sc")  # [128, 512]

        def tail_half(tc_):
            # softmax + context for t in [tc_*128, tc_*128+128): score cols done.
            e_out = e_exp[:].rearrange("p (kh t) -> p t kh", kh=2)[:, tc_ * 128 : (tc_ + 1) * 128, :]
            sc_in = score_ps[:].rearrange("p (t kh) -> p t kh", kh=2)[:, tc_ * 128 : (tc_ + 1) * 128, :]
            nc.scalar.activation(e_out, sc_in, AF.Exp)
            sum_ps = ps_sum.tile([128, 1], F32, tag="sum")
            for kh in range(2):
                nc.tensor.matmul(
                    sum_ps[:],
                    e_exp[:, kh * 256 + tc_ * 128 : kh * 256 + tc_ * 128 + 128],
                    ones_sb[:],
                    start=(kh == 0),
                    stop=(kh == 1),
                )
            rs = cpool.tile([128, 1], F32, tag=f"rs{tc_}")
            nc.vector.reciprocal(rs[:], sum_ps[:])
            ctx_ps = ps_ek.tile([128, D], F32, tag="ek")
            for kh in range(2):
                nc.tensor.matmul(
                    ctx_ps[:],
                    e_exp[:, kh * 256 + tc_ * 128 : kh * 256 + tc_ * 128 + 128],
                    know_sb[:, kh * D : (kh + 1) * D],
                    start=(kh == 0),
                    stop=(kh == 1),
                )
            o_sb = cpool.tile([128, D], F32, tag=f"osb{tc_}")
            nc.vector.tensor_scalar_mul(o_sb[:], ctx_ps[:], rs[:, 0:1])
            nc.sync.dma_start(out[tc_ * 128 : (tc_ + 1) * 128, :], o_sb[:])

        half_blk = NBLK // 2
        for r in range(reps):
            for blk in range(NBLK):
                h = hpool.tile([128, TBLK * K], BF16, tag="h")
                for j in range(TBLK):
                    t = blk * TBLK + j
                    nc.vector.tensor_scalar_add(h[:, j * K : (j + 1) * K], k_bf[:], e_sb[:, t : t + 1])
                hb = hbpool.tile([128, TBLK * K], BF16, tag="hb")
                half = TBLK * K // 2
                nc.scalar.activation(hb[:, 0:half], h[:, 0:half], AF.Tanh, bias=b12[:])
                nc.scalar.activation(hb[:, half:], h[:, half:], AF.Tanh, bias=b12[:])
                for i in range(2 * TBLK):
                    col = blk * 2 * TBLK + i
                    nc.tensor.matmul(
                        score_ps[:, col : col + 1],
                        hb[:, i * 128 : (i + 1) * 128],
                        v_bf[:],
                        start=True,
                        stop=True,
                    )
                if r == reps - 1 and blk == half_blk - 1:
                    tail_half(0)
            if r == reps - 1:
                tail_half(1)


def build_program(reps=1):
    nc = bacc.Bacc("TRN2", target_bir_lowering=False, debug=False)
    enc = nc.dram_tensor("enc", [TL, D], F32, kind="ExternalInput")
    know = nc.dram_tensor("know", [K, D], F32, kind="ExternalInput")
    w1 = nc.dram_tensor("w1", [D, U], F32, kind="ExternalInput")
    b1 = nc.dram_tensor("b1", [U, 1], F32, kind="ExternalInput")
    w2 = nc.dram_tensor("w2", [D, U], F32, kind="ExternalInput")
    b2 = nc.dram_tensor("b2", [U, 1], F32, kind="ExternalInput")
    v = nc.dram_tensor("v", [U, 1], F32, kind="ExternalInput")
    out = nc.dram_tensor("out", [TL, D], F32, kind="ExternalOutput")
    with tile.TileContext(nc) as tc:
        _body(tc, enc, know, w1, b1, w2, b2, v, out, reps=reps)
    nc.compile()
    return nc


def make_in_maps(inputs):
    f = lambda a: np.ascontiguousarray(np.asarray(a, dtype=np.float32))
    know = f(inputs["knowledge_onehot"])
    enc = f(inputs["encoder_outputs"])
    w1 = f(inputs["W1"])
    b1 = f(inputs["b1"]).reshape(U, 1)
    w2 = f(inputs["W2"])
    b2 = f(inputs["b2"]).reshape(U, 1)
    v = f(inputs["V"]).reshape(U, 1)
    in_maps = []
    for c in range(NCORES):
        b, th = divmod(c, 2)
        in_maps.append(
            {
                "enc": np.ascontiguousarray(enc[b, th * TL : (th + 1) * TL, :]),
                "know": np.ascontiguousarray(know[b]),
                "w1": w1,
                "b1": b1,
                "w2": w2,
                "b2": b2,
                "v": v,
            }
        )
    return in_maps


def run(inputs, trace=False, trace_kwargs=None):
    nc = build_program()
    in_maps = make_in_maps(inputs)
    res = run_bass_kernel_spmd(
        nc, in_maps, list(range(NCORES)), trace=trace, **(trace_kwargs or {})
    )
    out = np.empty((B, T, D), dtype=np.float32)
    for c in range(NCORES):
        b, th = divmod(c, 2)
        out[b, th * TL : (th + 1) * TL, :] = res.results[c]["out"]
    return out, res


def kernel(**inputs) -> np.ndarray:
    out, _ = run(inputs, trace=False)
    return out
